# revision 7
# baseline (speedup 1.0000x reference)
"""Trainium2 Bass kernel for nn_EpisodeMultiheadAttentionBlock.

Data-parallel over batch: each of the 8 NeuronCores handles one batch element
(B=8). Per core, a fused attention block:

  q/k/v projections (f32r matmuls) -> causal+pad+eye masked attention with a
  max-free softmax (scores bounded), computed in BOTH [q,k] and [k,q]
  orientations to avoid on-device transposes -> context -> out projection ->
  LayerNorm -> residual.  attn_weights (head-mean of probs) accumulate on the
  PE via diag(1/(H*l)) matmuls.

Masking is additive (-2^96) built from:
  - rank-1 K=1 matmuls for the key-padding mask / fully-masked blocks
  - host-precomputed [128,128] diagonal-block masks (causal+pad+eye) added on
    the vector engine directly into PSUM.
Causal structure skips fully-masked score blocks entirely.
"""
import sys

if "/opt/trn_rl_repo" not in sys.path:
    sys.path.insert(0, "/opt/trn_rl_repo")

import numpy as np
import ml_dtypes

import concourse.bass as bass
import concourse.tile as tile
from concourse import bacc, mybir
from concourse.bass_utils import run_bass_kernel_spmd

F32 = mybir.dt.float32
F32R = mybir.dt.float32r
BF16 = mybir.dt.bfloat16
Act = mybir.ActivationFunctionType

B = 8
L = 1024
E = 1024
H = 16
D = E // H          # 64
P = 128
NT = L // P         # 8
NE = E // P         # 8
HP = H // 2         # head pairs
BIG = float(2 ** 96)
LN_EPS = 1e-5
SCALE = 1.0 / np.sqrt(D)  # 0.125


def _chunks(start, end, step=512):
    out = []
    while start < end:
        out.append((start, min(start + step, end)))
        start += step
    return out


def build():
    nc = bacc.Bacc("TRN2", target_bir_lowering=False, debug=False, num_devices=B)

    xt_d = nc.dram_tensor("xt", [E, L], F32R, kind="ExternalInput").ap()
    xres_d = nc.dram_tensor("xres", [L, E], F32, kind="ExternalInput").ap()
    wq_d = nc.dram_tensor("wq", [E, E], F32R, kind="ExternalInput").ap()
    wk_d = nc.dram_tensor("wk", [E, E], F32R, kind="ExternalInput").ap()
    wv_d = nc.dram_tensor("wv", [E, E], F32R, kind="ExternalInput").ap()
    wo_d = nc.dram_tensor("wo", [E, E], F32R, kind="ExternalInput").ap()
    bq_d = nc.dram_tensor("bq", [E], BF16, kind="ExternalInput").ap()
    bk_d = nc.dram_tensor("bk", [E], BF16, kind="ExternalInput").ap()
    bv_d = nc.dram_tensor("bv", [E], BF16, kind="ExternalInput").ap()
    bo_d = nc.dram_tensor("bo", [E], BF16, kind="ExternalInput").ap()
    pad_d = nc.dram_tensor("pad", [L], BF16, kind="ExternalInput").ap()
    madd_d = nc.dram_tensor("madd", [NT, P, P], F32, kind="ExternalInput").ap()
    maddt_d = nc.dram_tensor("maddt", [NT, P, P], F32, kind="ExternalInput").ap()
    g_d = nc.dram_tensor("g", [E], F32, kind="ExternalInput").ap()

    out_d = nc.dram_tensor("out", [L, E], F32, kind="ExternalOutput").ap()
    attn_d = nc.dram_tensor("attn", [L, L], F32, kind="ExternalOutput").ap()

    with tile.TileContext(nc) as tc:
        # ---------- long-lived constants ----------
        with (
            tc.tile_pool(name="consts", bufs=1) as consts,
            tc.tile_pool(name="dscratch", bufs=1, space="DRAM") as dscratch,
        ):
            sdram = dscratch.tile([H, L], F32)
            ctxt_d = dscratch.tile([E, L], F32R)
            ones_bf = consts.tile([1, L], BF16)
            nc.vector.memset(ones_bf[:], 1.0)
            negbig = consts.tile([1, P], BF16)
            nc.vector.memset(negbig[:], -BIG)
            pad_sb = consts.tile([1, L], BF16)
            nc.sync.dma_start(out=pad_sb[:], in_=pad_d.rearrange("(o n) -> o n", o=1))
            bq_sb = consts.tile([1, E], BF16)
            nc.sync.dma_start(out=bq_sb[:], in_=bq_d.rearrange("(o n) -> o n", o=1))
            bk_sb = consts.tile([1, E], BF16)
            nc.sync.dma_start(out=bk_sb[:], in_=bk_d.rearrange("(o n) -> o n", o=1))
            bv_sb = consts.tile([1, E], BF16)
            nc.sync.dma_start(out=bv_sb[:], in_=bv_d.rearrange("(o n) -> o n", o=1))
            bo_sb = consts.tile([1, E], BF16)
            nc.sync.dma_start(out=bo_sb[:], in_=bo_d.rearrange("(o n) -> o n", o=1))
            madd_sb = consts.tile([P, NT, P], F32)
            nc.sync.dma_start(out=madd_sb[:], in_=madd_d.rearrange("t p j -> p t j"))
            maddt_sb = consts.tile([P, NT, P], F32)
            nc.sync.dma_start(out=maddt_sb[:], in_=maddt_d.rearrange("t p j -> p t j"))
            g_bcast = consts.tile([P, E], F32)
            nc.sync.dma_start(
                out=g_bcast[:],
                in_=bass.AP(tensor=g_d.tensor, offset=0, ap=[[0, P], [1, E]]),
            )
            eps_sb = consts.tile([P, 1], F32)
            nc.vector.memset(eps_sb[:], LN_EPS)
            idn = consts.tile([P, P], BF16)
            nc.vector.memset(idn[:], 1.0)
            idnm = consts.tile([P, P], BF16)
            nc.gpsimd.affine_select(
                out=idnm[:], in_=idn[:],
                pattern=[[-1, P]], base=0, channel_multiplier=1,
                compare_op=mybir.AluOpType.is_equal, fill=0.0,
            )

            # ---------- persistent activations ----------
            with tc.tile_pool(name="acts", bufs=1) as acts:
                qt_sb = acts.tile([P, NE, L], F32R)   # [e' in tile, me, t]
                kt_sb = acts.tile([P, NE, L], F32R)
                v_sb = acts.tile([P, NT, E], BF16)    # [t in tile, mt, e']

                # ================= phase 1: projections =================
                with (
                    tc.tile_pool(name="p1", bufs=1) as p1,
                    tc.tile_pool(name="wstr", bufs=6) as wstr,
                    tc.tile_pool(name="ps1", bufs=4, space="PSUM") as ps1,
                ):
                    xt_sb = p1.tile([P, NE, L], F32R)
                    nc.sync.dma_start(
                        out=xt_sb[:], in_=xt_d.rearrange("(ke p) t -> p ke t", p=P)
                    )
                    wv_sb = p1.tile([P, NE, E], F32R)
                    nc.sync.dma_start(
                        out=wv_sb[:], in_=wv_d.rearrange("(ke p) e -> p ke e", p=P)
                    )

                    for w_d, b_sb, dst in ((wq_d, bq_sb, qt_sb), (wk_d, bk_sb, kt_sb)):
                        for me in range(NE):
                            psc = [
                                ps1.tile([P, 512], F32, name=f"psqk{me}c{c}", tag=f"psqk{c}")
                                for c in range(2)
                            ]
                            for ke in range(NE):
                                wt = wstr.tile([P, P], F32R, name=f"wt{me}k{ke}", tag="wt")
                                nc.sync.dma_start(
                                    out=wt[:],
                                    in_=w_d[ke * P:(ke + 1) * P, me * P:(me + 1) * P],
                                )
                                for c in range(2):
                                    nc.tensor.matmul(
                                        psc[c][:], wt[:], xt_sb[:, ke, c * 512:(c + 1) * 512],
                                        start=(ke == 0), stop=False,
                                    )
                            for c in range(2):
                                nc.tensor.matmul(
                                    psc[c][:],
                                    b_sb[0:1, me * P:(me + 1) * P],
                                    ones_bf[0:1, 0:512],
                                    start=False, stop=True,
                                )
                                nc.scalar.copy(
                                    out=dst[:, me, c * 512:(c + 1) * 512], in_=psc[c][:]
                                )

                    for mt in range(NT):
                        psc = [
                            ps1.tile([P, 512], F32, name=f"psv{mt}c{c}", tag=f"psqk{c}")
                            for c in range(2)
                        ]
                        for ke in range(NE):
                            for c in range(2):
                                nc.tensor.matmul(
                                    psc[c][:],
                                    xt_sb[:, ke, mt * P:(mt + 1) * P],
                                    wv_sb[:, ke, c * 512:(c + 1) * 512],
                                    start=(ke == 0), stop=False,
                                )
                        for c in range(2):
                            nc.tensor.matmul(
                                psc[c][:],
                                ones_bf[0:1, 0:P],
                                bv_sb[0:1, c * 512:(c + 1) * 512],
                                start=False, stop=True,
                            )
                            nc.scalar.copy(
                                out=v_sb[:, mt, c * 512:(c + 1) * 512], in_=psc[c][:]
                            )

                # ================= phase 2a: A-path ([q,k]) =================
                with (
                    tc.tile_pool(name="p2a", bufs=1) as p2a,
                    tc.tile_pool(name="pexp", bufs=3) as pexp,
                    tc.tile_pool(name="small", bufs=8) as small,
                    tc.tile_pool(name="aout", bufs=2) as aoutp,
                    tc.tile_pool(name="psSm", bufs=2, space="PSUM") as psSm,
                    tc.tile_pool(name="psSd", bufs=2, space="PSUM") as psSd,
                    tc.tile_pool(name="psA", bufs=1, space="PSUM") as psA,
                ):
                    s_all = p2a.tile([P, H, NT], F32)
                    zeros = p2a.tile([P, L - P], F32)
                    nc.vector.memset(zeros[:], 0.0)
                    for qt in range(NT - 1):
                        nc.sync.dma_start(
                            out=attn_d[qt * P:(qt + 1) * P, (qt + 1) * P:L],
                            in_=zeros[:, 0:L - (qt + 1) * P],
                        )
                    for qt in range(NT):
                        W = (qt + 1) * P
                        dc = qt * P          # diag block start
                        a_ps = psA.tile([P, L], F32, name=f"aps{qt}", tag="aps")
                        for h in range(H):
                            po = (h % 2) * 64
                            qslice = qt_sb[po:po + 64, h // 2, qt * P:(qt + 1) * P]
                            p_t = pexp.tile([P, L], BF16, name=f"pt{qt}h{h}", tag="pt")
                            l_t = small.tile([P, 2], F32, name=f"lt{qt}h{h}", tag="lt")
                            if dc > 0:
                                s_mn = psSm.tile(
                                    [P, 896], F32, name=f"smn{qt}h{h}", tag="smn"
                                )
                                for (cs, ce) in _chunks(0, dc):
                                    nc.tensor.matmul(
                                        s_mn[:, cs:ce], qslice,
                                        kt_sb[po:po + 64, h // 2, cs:ce],
                                        start=True, stop=False,
                                    )
                                    nc.tensor.matmul(
                                        s_mn[:, cs:ce],
                                        ones_bf[0:1, 0:P],
                                        pad_sb[0:1, cs:ce],
                                        start=False, stop=True,
                                    )
                                nc.scalar.activation(
                                    out=p_t[:, 0:dc], in_=s_mn[:, 0:dc],
                                    func=Act.Exp, scale=SCALE,
                                    accum_out=l_t[:, 0:1],
                                )
                            else:
                                nc.vector.memset(l_t[:, 0:1], 0.0)
                            s_dg = psSd.tile([P, P], F32, name=f"sdg{qt}h{h}", tag="sdg")
                            nc.tensor.matmul(
                                s_dg[:], qslice,
                                kt_sb[po:po + 64, h // 2, dc:W],
                                start=True, stop=True,
                            )
                            nc.vector.tensor_add(
                                out=s_dg[:], in0=s_dg[:], in1=madd_sb[:, qt, :]
                            )
                            nc.scalar.activation(
                                out=p_t[:, dc:W], in_=s_dg[:],
                                func=Act.Exp, scale=SCALE,
                                accum_out=l_t[:, 1:2],
                            )
                            l_s = small.tile([P, 1], F32, name=f"ls{qt}h{h}", tag="ls")
                            nc.vector.tensor_reduce(
                                out=l_s[:], in_=l_t[:],
                                axis=mybir.AxisListType.X, op=mybir.AluOpType.add,
                            )
                            l16 = small.tile([P, 1], F32, name=f"l16{qt}h{h}", tag="l16")
                            nc.vector.tensor_scalar_mul(l16[:], l_s[:], 16.0)
                            s_col = small.tile([P, 1], F32, name=f"sc{qt}h{h}", tag="sc")
                            nc.vector.reciprocal(out=s_col[:], in_=l16[:])
                            nc.vector.tensor_copy(
                                out=s_all[:, h, qt:qt + 1], in_=s_col[:]
                            )
                            dg = small.tile([P, P], BF16, name=f"dg{qt}h{h}", tag="dg")
                            nc.vector.tensor_scalar_mul(dg[:], idnm[:], s_col[:])
                            for (cs, ce) in _chunks(0, W):
                                nc.tensor.matmul(
                                    a_ps[:, cs:ce], dg[:], p_t[:, cs:ce],
                                    start=(h == 0), stop=(h == H - 1),
                                )
                        a_out = aoutp.tile([P, L], F32, name=f"aout{qt}", tag="aout")
                        nc.scalar.copy(out=a_out[:, 0:W], in_=a_ps[:, 0:W])
                        nc.sync.dma_start(
                            out=attn_d[qt * P:(qt + 1) * P, 0:W], in_=a_out[:, 0:W]
                        )
                    nc.sync.dma_start(
                        out=bass.AP(
                            tensor=sdram.tensor, offset=sdram.offset,
                            ap=[[1, P], [L, H], [P, NT]],
                        ),
                        in_=s_all[:],
                    )

                # ================= phase 2b: ST/ctx path ([k,q]) =================
                with (
                    tc.tile_pool(name="ptp", bufs=3) as ptp,
                    tc.tile_pool(name="sbcp", bufs=2) as sbcp,
                    tc.tile_pool(name="coutp", bufs=2) as coutp,
                    tc.tile_pool(name="psTd", bufs=2, space="PSUM") as psTd,
                    tc.tile_pool(name="psTp", bufs=2, space="PSUM") as psTp,
                    tc.tile_pool(name="psC", bufs=1, space="PSUM") as psC,
                ):
                    for gp in range(HP):
                        ctx_ps = psC.tile([P, L], F32, name=f"ctxps{gp}", tag="ctxps")
                        for h in (2 * gp, 2 * gp + 1):
                            po = (h % 2) * 64
                            for kt in range(NT):
                                c0 = 512 * (kt // 4)
                                kslice = kt_sb[po:po + 64, h // 2, kt * P:(kt + 1) * P]
                                pt_t = ptp.tile(
                                    [P, L], BF16, name=f"ptt{gp}h{h}k{kt}", tag="ptt"
                                )
                                if c0 < kt * P:
                                    # fully-masked region: probs are exactly 0
                                    nc.vector.memset(pt_t[:, c0:kt * P], 0.0)
                                st_dg = psTd.tile(
                                    [P, P], F32, name=f"std{gp}h{h}k{kt}", tag="std"
                                )
                                nc.tensor.matmul(
                                    st_dg[:], kslice,
                                    qt_sb[po:po + 64, h // 2, kt * P:(kt + 1) * P],
                                    start=True, stop=True,
                                )
                                nc.vector.tensor_add(
                                    out=st_dg[:], in0=st_dg[:], in1=maddt_sb[:, kt, :]
                                )
                                nc.scalar.activation(
                                    out=pt_t[:, kt * P:(kt + 1) * P], in_=st_dg[:],
                                    func=Act.Exp, scale=SCALE,
                                )
                                ps_ = (kt + 1) * P  # pad region start
                                if ps_ < L:
                                    st_pd = psTp.tile(
                                        [P, 896], F32, name=f"stp{gp}h{h}k{kt}", tag="stp"
                                    )
                                    for (cs, ce) in _chunks(ps_, L):
                                        nc.tensor.matmul(
                                            st_pd[:, cs - ps_:ce - ps_], kslice,
                                            qt_sb[po:po + 64, h // 2, cs:ce],
                                            start=True, stop=False,
                                        )
                                        nc.tensor.matmul(
                                            st_pd[:, cs - ps_:ce - ps_],
                                            pad_sb[0:1, kt * P:(kt + 1) * P],
                                            ones_bf[0:1, cs:ce],
                                            start=False, stop=True,
                                        )
                                    nc.scalar.activation(
                                        out=pt_t[:, ps_:L], in_=st_pd[:, 0:L - ps_],
                                        func=Act.Exp, scale=SCALE,
                                    )
                                for (cs, ce) in _chunks(c0, L):
                                    n_kt = min(NT, (ce + P - 1) // P)
                                    nc.tensor.matmul(
                                        ctx_ps[po:po + 64, cs:ce],
                                        v_sb[:, kt, h * D:(h + 1) * D],
                                        pt_t[:, cs:ce],
                                        start=(kt == 0),
                                        stop=(kt == n_kt - 1),
                                    )
                        sbc = sbcp.tile([P, L], F32, name=f"sbc{gp}", tag="sbc")
                        nc.sync.dma_start(
                            out=sbc[:],
                            in_=bass.AP(
                                tensor=sdram.tensor, offset=sdram.offset + 2 * gp * L,
                                ap=[[L, 2], [0, 64], [1, L]],
                            ),
                        )
                        cout = coutp.tile([P, L], F32R, name=f"cout{gp}", tag="cout")
                        nc.vector.scalar_tensor_tensor(
                            out=cout[:], in0=ctx_ps[:], scalar=16.0, in1=sbc[:],
                            op0=mybir.AluOpType.mult, op1=mybir.AluOpType.mult,
                        )
                        nc.sync.dma_start(
                            out=ctxt_d[gp * P:(gp + 1) * P, :], in_=cout[:]
                        )

            # ================= phase 3: out-proj + LN + residual =================
            with (
                tc.tile_pool(name="p3", bufs=1) as p3,
                tc.tile_pool(name="xrp", bufs=2) as xrp,
                tc.tile_pool(name="znp", bufs=2) as znp,
                tc.tile_pool(name="lns", bufs=8) as lns,
                tc.tile_pool(name="ps3", bufs=4, space="PSUM") as ps3,
            ):
                ctxt_sb = p3.tile([P, NE, L], F32R)
                nc.sync.dma_start(
                    out=ctxt_sb[:], in_=ctxt_d.rearrange("(ke p) t -> p ke t", p=P)
                )
                wo_sb = p3.tile([P, NE, E], F32R)
                nc.sync.dma_start(
                    out=wo_sb[:], in_=wo_d.rearrange("(ke p) e -> p ke e", p=P)
                )
                for qt in range(NT):
                    psc = [
                        ps3.tile([P, 512], F32, name=f"pso{qt}c{c}", tag=f"pso{c}")
                        for c in range(2)
                    ]
                    for ke in range(NE):
                        for c in range(2):
                            nc.tensor.matmul(
                                psc[c][:],
                                ctxt_sb[:, ke, qt * P:(qt + 1) * P],
                                wo_sb[:, ke, c * 512:(c + 1) * 512],
                                start=(ke == 0), stop=False,
                            )
                    for c in range(2):
                        nc.tensor.matmul(
                            psc[c][:],
                            ones_bf[0:1, 0:P],
                            bo_sb[0:1, c * 512:(c + 1) * 512],
                            start=False, stop=True,
                        )
                    stats = lns.tile([P, 2, 6], F32, name=f"st{qt}", tag="st")
                    for c in range(2):
                        nc.vector.bn_stats(out=stats[:, c, :], in_=psc[c][:])
                    mv = lns.tile([P, 2], F32, name=f"mv{qt}", tag="mv")
                    nc.vector.bn_aggr(out=mv[:], in_=stats[:])
                    sd = lns.tile([P, 1], F32, name=f"sd{qt}", tag="sd")
                    nc.scalar.activation(
                        out=sd[:], in_=mv[:, 1:2], func=Act.Sqrt, bias=eps_sb[:],
                    )
                    rstd = lns.tile([P, 1], F32, name=f"rs{qt}", tag="rs")
                    nc.vector.reciprocal(out=rstd[:], in_=sd[:])
                    nmu = lns.tile([P, 1], F32, name=f"nm{qt}", tag="nm")
                    nc.vector.scalar_tensor_tensor(
                        out=nmu[:], in0=mv[:, 0:1], scalar=-1.0, in1=rstd[:],
                        op0=mybir.AluOpType.mult, op1=mybir.AluOpType.mult,
                    )
                    zn = znp.tile([P, E], F32, name=f"zn{qt}", tag="zn")
                    for c in range(2):
                        nc.scalar.activation(
                            out=zn[:, c * 512:(c + 1) * 512], in_=psc[c][:],
                            func=Act.Identity, bias=nmu[:], scale=rstd[:],
                        )
                    xr = xrp.tile([P, E], F32, name=f"xr{qt}", tag="xr")
                    nc.sync.dma_start(out=xr[:], in_=xres_d[qt * P:(qt + 1) * P, :])
                    nc.vector.tensor_mul(zn[:], zn[:], g_bcast[:])
                    nc.vector.tensor_add(zn[:], zn[:], xr[:])
                    nc.sync.dma_start(out=out_d[qt * P:(qt + 1) * P, :], in_=zn[:])

    nc.compile()
    return nc


_NC = None


def _get_nc():
    global _NC
    if _NC is None:
        _NC = build()
    return _NC


def _host_prep(key, key_padding_mask, in_proj_w, in_proj_b, out_w, out_b, ln_g, ln_b):
    key = np.asarray(key, np.float32)
    mask = np.asarray(key_padding_mask).astype(bool)
    in_proj_w = np.asarray(in_proj_w, np.float32)
    in_proj_b = np.asarray(in_proj_b, np.float32)
    out_w = np.asarray(out_w, np.float32)
    out_b = np.asarray(out_b, np.float32)
    ln_g = np.asarray(ln_g, np.float32)
    ln_b = np.asarray(ln_b, np.float32)

    wq = np.ascontiguousarray(in_proj_w[:E].T)
    wk = np.ascontiguousarray(in_proj_w[E:2 * E].T)
    wv = np.ascontiguousarray(in_proj_w[2 * E:].T)
    wo = np.ascontiguousarray(out_w.T)
    bq = in_proj_b[:E].astype(ml_dtypes.bfloat16)
    bk = in_proj_b[E:2 * E].astype(ml_dtypes.bfloat16)
    bv = in_proj_b[2 * E:].astype(ml_dtypes.bfloat16)
    bo = out_b.astype(ml_dtypes.bfloat16)

    # diag-block masks: q = qt*P+p, k = qt*P+j
    jj = np.arange(P)
    tri = jj[None, :] > jj[:, None]               # j > p  (causal, strict)
    eye = jj[None, :] == jj[:, None]
    in_maps = []
    for b in range(B):
        pad_row = np.where(mask[b], -BIG, 0.0).astype(ml_dtypes.bfloat16)
        madd = np.zeros((NT, P, P), np.float32)
        for qt in range(NT):
            pm = mask[b, qt * P:(qt + 1) * P][None, :]  # pad of k within block
            m = tri | (pm & ~eye)
            madd[qt][m] = -BIG
        maddt = np.ascontiguousarray(np.transpose(madd, (0, 2, 1)))
        in_maps.append({
            "xt": np.ascontiguousarray(key[b].T),
            "xres": np.ascontiguousarray(key[b] + ln_b[None, :]),
            "wq": wq, "wk": wk, "wv": wv, "wo": wo,
            "bq": bq, "bk": bk, "bv": bv, "bo": bo,
            "pad": pad_row,
            "madd": madd, "maddt": maddt,
            "g": ln_g,
        })
    return in_maps


def kernel(key, query_length, key_padding_mask, in_proj_w, in_proj_b,
           out_w, out_b, ln_g, ln_b):
    assert int(query_length) == L
    nc = _get_nc()
    in_maps = _host_prep(key, key_padding_mask, in_proj_w, in_proj_b,
                         out_w, out_b, ln_g, ln_b)
    res = run_bass_kernel_spmd(nc, in_maps, core_ids=list(range(B)))
    out = np.stack([res.results[b]["out"] for b in range(B)])
    attn = np.stack([res.results[b]["attn"] for b in range(B)])
    return out, attn
